# revision 7
# baseline (speedup 1.0000x reference)
"""Self-attention (CrossAttention module with q=k=v=x) kernel for Trainium2.

Problem: x [B=4, N=4096, H=256] fp32; Wq/Wk/Wv [256,256], bq/bk/bv [256].
  q = x@Wq.T+bq ; k = x@Wk.T+bk ; v = x@Wv.T+bv
  out = softmax(q@k.T) @ v          (no 1/sqrt(d) scaling)

Sharding: 8 cores = batch (4) x query-halves (2). Each core holds the full
K/V sequence for its batch element and 2048 query rows.

Per-core layout strategy (all matmuls in float32r = TF32 on the PE, fp32
accumulation):
  - Host pre-transposes x -> xT [H, N] and weights -> W.T [h, o] so that
    Q^T/K^T land in [o, n] layout directly off the projection matmuls.
  - scores are computed TRANSPOSED: S_T[j, i] = sum_o K_T[o,j] * Q_T[o,i]
    (K_T chunk stationary, Q_T moving), so exp(S_T) feeds the AV matmul
    as the stationary operand with no on-chip transpose.
  - V is produced in natural [n, o] layout with a 257th column of ones:
    att_psum[i, 0:256] = sum_j P_T[j,i] V[j,:], att_psum[i, 256] = row sum
    of P (the softmax denominator) -- the denominator comes free.
  - normalize with DVE reciprocal + per-partition broadcast multiply.
  - v-bias is added on the host after gathering (softmax rows sum to 1).
"""

import sys

import numpy as np

if "/opt/trn_rl_repo" not in sys.path:
    sys.path.insert(0, "/opt/trn_rl_repo")

B, N, H = 4, 4096, 256
P = 128
NQ = N // 2  # query rows per core


def build_nc(nkv=N, nq=NQ, iblk=512):
    import concourse.mybir as mybir
    import concourse.tile as tile
    from concourse import bacc

    f32 = mybir.dt.float32
    FR = mybir.dt.float16   # projections + scores matmul dtype (11-bit mantissa)
    AVT = mybir.dt.bfloat16  # exp(S) and V dtype: needs fp32-like range
    Exp = mybir.ActivationFunctionType.Exp

    assert nkv % 512 == 0 and nq % iblk == 0 and iblk % P == 0 and iblk <= 512
    JC = nkv // P          # key chunks
    ICH = iblk // P        # query sub-chunks per block
    NBLK = nq // iblk      # query blocks
    kseg = 512             # K_T projection segment (moving free dim)
    qseg = min(512, nq)

    nc = bacc.Bacc("TRN2", target_bir_lowering=False, debug=False)

    xT_d = nc.dram_tensor("xT", [H, nkv], f32, kind="ExternalInput").ap()
    xqT_d = nc.dram_tensor("xqT", [H, nq], f32, kind="ExternalInput").ap()
    wqT_d = nc.dram_tensor("wqT", [H, H], f32, kind="ExternalInput").ap()
    wkT_d = nc.dram_tensor("wkT", [H, H], f32, kind="ExternalInput").ap()
    wvT_d = nc.dram_tensor("wvT", [H, H], f32, kind="ExternalInput").ap()
    bq_d = nc.dram_tensor("bq", [H, 1], f32, kind="ExternalInput").ap()
    bk_d = nc.dram_tensor("bk", [H, 1], f32, kind="ExternalInput").ap()
    att_d = nc.dram_tensor("att", [nq, H], f32, kind="ExternalOutput").ap()

    with tile.TileContext(nc) as tc:
        with tc.tile_pool(name="io", bufs=1) as io, \
             tc.tile_pool(name="kqv", bufs=1) as kqv, \
             tc.tile_pool(name="expp", bufs=6) as expp, \
             tc.tile_pool(name="op", bufs=4) as op, \
             tc.tile_pool(name="psmm", bufs=3, space="PSUM") as psmm, \
             tc.tile_pool(name="psatt", bufs=4, space="PSUM") as psatt:

            # ---- input loads (gpsimd DMA casts f32 -> f32r on the fly) ----
            xt = [io.tile([P, nkv], FR, tag=f"xt{h}", name=f"xt{h}") for h in range(2)]
            xq = [io.tile([P, nq], FR, tag=f"xq{h}", name=f"xq{h}") for h in range(2)]
            wq = [io.tile([P, H], FR, tag=f"wq{h}", name=f"wq{h}") for h in range(2)]
            wk = [io.tile([P, H], FR, tag=f"wk{h}", name=f"wk{h}") for h in range(2)]
            wv = [io.tile([P, H], FR, tag=f"wv{h}", name=f"wv{h}") for h in range(2)]
            bqt = [io.tile([P, 1], f32, tag=f"bq{h}", name=f"bq{h}") for h in range(2)]
            bkt = [io.tile([P, 1], f32, tag=f"bk{h}", name=f"bk{h}") for h in range(2)]
            for h in range(2):
                hs = slice(h * P, (h + 1) * P)
                nc.gpsimd.dma_start(xt[h][:], xT_d[hs, :])
                nc.gpsimd.dma_start(xq[h][:], xqT_d[hs, :])
                nc.gpsimd.dma_start(wq[h][:], wqT_d[hs, :])
                nc.gpsimd.dma_start(wk[h][:], wkT_d[hs, :])
                nc.gpsimd.dma_start(wv[h][:], wvT_d[hs, :])
                nc.sync.dma_start(bqt[h][:], bq_d[hs, :])
                nc.sync.dma_start(bkt[h][:], bk_d[hs, :])

            # ---- projections ----
            # K_T[o, j] = sum_h WkT[h, o] * xT[h, j]   (+ bk[o])
            kT = [kqv.tile([P, nkv], FR, tag=f"kT{oc}", name=f"kT{oc}") for oc in range(2)]
            qT = [kqv.tile([P, nq], FR, tag=f"qT{oc}", name=f"qT{oc}") for oc in range(2)]
            for oc in range(2):
                ocs = slice(oc * P, (oc + 1) * P)
                for s in range(nkv // kseg):
                    ss = slice(s * kseg, (s + 1) * kseg)
                    pk = psmm.tile([P, kseg], f32, tag="mm", name="pk")
                    for h in range(2):
                        nc.tensor.matmul(pk[:], wk[h][:, ocs], xt[h][:, ss],
                                         start=(h == 0), stop=(h == 1))
                    nc.vector.tensor_scalar_add(kT[oc][:, ss], pk[:], bkt[oc][:])
                for s in range(nq // qseg):
                    ss = slice(s * qseg, (s + 1) * qseg)
                    pq = psmm.tile([P, qseg], f32, tag="mm", name="pq")
                    for h in range(2):
                        nc.tensor.matmul(pq[:], wq[h][:, ocs], xq[h][:, ss],
                                         start=(h == 0), stop=(h == 1))
                    nc.vector.tensor_scalar_add(qT[oc][:, ss], pq[:], bqt[oc][:])

            # V[n, o] = sum_h xT[h, n] * WvT[h, o]  (no bias: added on host),
            # plus two trailing columns of ones: column H yields the softmax
            # denominator off the AV matmul; two of them because fp32r
            # matmuls require an even moving free-dim count.
            ones2 = io.tile([P, 2], f32, tag="ones2", name="ones2")
            nc.vector.memset(ones2[:], 1.0)
            vt = [kqv.tile([P, H + 2], AVT, tag=f"v{j}", name=f"v{j}") for j in range(JC)]
            for j in range(JC):
                js = slice(j * P, (j + 1) * P)
                pv = psmm.tile([P, H], f32, tag="mm", name="pv")
                for h in range(2):
                    nc.tensor.matmul(pv[:], xt[h][:, js], wv[h][:],
                                     start=(h == 0), stop=(h == 1))
                nc.vector.tensor_copy(vt[j][:, 0:H], pv[:])
                nc.vector.tensor_copy(vt[j][:, H:H + 2], ones2[:])

            # ---- attention blocks ----
            for blk in range(NBLK):
                bs = slice(blk * iblk, (blk + 1) * iblk)
                att_ps = [psatt.tile([P, H + 2], f32, tag="att", name="attps")
                          for _ in range(ICH)]
                for jc in range(JC):
                    jcs = slice(jc * P, (jc + 1) * P)
                    sc = psmm.tile([P, iblk], f32, tag="mm", name="sc")
                    for oc in range(2):
                        nc.tensor.matmul(sc[:], kT[oc][:, jcs], qT[oc][:, bs],
                                         start=(oc == 0), stop=(oc == 1))
                    ex = expp.tile([P, iblk], AVT, tag="ex", name="ex")
                    nc.scalar.activation(ex[:], sc[:], Exp)
                    for ic in range(ICH):
                        ics = slice(ic * P, (ic + 1) * P)
                        nc.tensor.matmul(att_ps[ic][:], ex[:, ics], vt[jc][:],
                                         start=(jc == 0), stop=(jc == JC - 1))
                for ic in range(ICH):
                    rec = op.tile([P, 1], f32, tag="rec", name="rec")
                    nc.vector.reciprocal(rec[:], att_ps[ic][:, H:H + 1])
                    ao = op.tile([P, H], f32, tag="ao", name="ao")
                    nc.vector.tensor_scalar_mul(ao[:], att_ps[ic][:, 0:H], rec[:])
                    r0 = blk * iblk + ic * P
                    nc.sync.dma_start(att_d[r0:r0 + P, :], ao[:])

    nc.compile()
    return nc


_NC_CACHE = {}


def _get_nc(nkv=N, nq=NQ, iblk=512):
    key = (nkv, nq, iblk)
    if key not in _NC_CACHE:
        _NC_CACHE[key] = build_nc(*key)
    return _NC_CACHE[key]


def _make_in_maps(x, Wq, bq, Wk, bk, Wv):
    wqT = np.ascontiguousarray(Wq.T)
    wkT = np.ascontiguousarray(Wk.T)
    wvT = np.ascontiguousarray(Wv.T)
    bq2 = np.ascontiguousarray(bq.reshape(H, 1))
    bk2 = np.ascontiguousarray(bk.reshape(H, 1))
    xT = [np.ascontiguousarray(x[b].T) for b in range(B)]
    in_maps = []
    for c in range(8):
        b, half = c // 2, c % 2
        in_maps.append({
            "xT": xT[b],
            "xqT": np.ascontiguousarray(x[b, half * NQ:(half + 1) * NQ, :].T),
            "wqT": wqT, "wkT": wkT, "wvT": wvT,
            "bq": bq2, "bk": bk2,
        })
    return in_maps


def _run(inputs, trace=False):
    from concourse.bass_utils import run_bass_kernel_spmd

    x = np.asarray(inputs["x"], dtype=np.float32)
    Wq = np.asarray(inputs["Wq"], dtype=np.float32)
    bq = np.asarray(inputs["bq"], dtype=np.float32)
    Wk = np.asarray(inputs["Wk"], dtype=np.float32)
    bk = np.asarray(inputs["bk"], dtype=np.float32)
    Wv = np.asarray(inputs["Wv"], dtype=np.float32)
    bv = np.asarray(inputs["bv"], dtype=np.float32)

    nc = _get_nc()
    in_maps = _make_in_maps(x, Wq, bq, Wk, bk, Wv)
    res = run_bass_kernel_spmd(nc, in_maps, list(range(8)), trace=trace)

    out = np.empty((B, N, H), dtype=np.float32)
    for c in range(8):
        b, half = c // 2, c % 2
        out[b, half * NQ:(half + 1) * NQ, :] = res.results[c]["att"] + bv
    return out, res


def kernel(**inputs) -> np.ndarray:
    out, _ = _run(inputs, trace=False)
    return out


# revision 8
# speedup vs baseline: 1.0302x; 1.0302x over previous
"""Self-attention (CrossAttention module with q=k=v=x) kernel for Trainium2.

Problem: x [B=4, N=4096, H=256] fp32; Wq/Wk/Wv [256,256], bq/bk/bv [256].
  q = x@Wq.T+bq ; k = x@Wk.T+bk ; v = x@Wv.T+bv
  out = softmax(q@k.T) @ v          (no 1/sqrt(d) scaling)

Sharding: 8 cores = batch (4) x query-halves (2). Each core holds the full
K/V sequence for its batch element and 2048 query rows.

Per-core layout strategy (all matmuls in float32r = TF32 on the PE, fp32
accumulation):
  - Host pre-transposes x -> xT [H, N] and weights -> W.T [h, o] so that
    Q^T/K^T land in [o, n] layout directly off the projection matmuls.
  - scores are computed TRANSPOSED: S_T[j, i] = sum_o K_T[o,j] * Q_T[o,i]
    (K_T chunk stationary, Q_T moving), so exp(S_T) feeds the AV matmul
    as the stationary operand with no on-chip transpose.
  - V is produced in natural [n, o] layout with a 257th column of ones:
    att_psum[i, 0:256] = sum_j P_T[j,i] V[j,:], att_psum[i, 256] = row sum
    of P (the softmax denominator) -- the denominator comes free.
  - normalize with DVE reciprocal + per-partition broadcast multiply.
  - v-bias is added on the host after gathering (softmax rows sum to 1).
"""

import sys

import numpy as np

if "/opt/trn_rl_repo" not in sys.path:
    sys.path.insert(0, "/opt/trn_rl_repo")

B, N, H = 4, 4096, 256
P = 128
NQ = N // 2  # query rows per core


def build_nc(nkv=N, nq=NQ, iblk=512):
    import concourse.mybir as mybir
    import concourse.tile as tile
    from concourse import bacc

    f32 = mybir.dt.float32
    FR = mybir.dt.float16   # projections + scores matmul dtype (11-bit mantissa)
    AVT = mybir.dt.bfloat16  # exp(S) and V dtype: needs fp32-like range
    Exp = mybir.ActivationFunctionType.Exp

    assert nkv % 512 == 0 and nq % iblk == 0 and iblk % P == 0 and iblk <= 512
    JC = nkv // P          # key chunks
    ICH = iblk // P        # query sub-chunks per block
    NBLK = nq // iblk      # query blocks
    kseg = 512             # K_T projection segment (moving free dim)
    qseg = min(512, nq)

    nc = bacc.Bacc("TRN2", target_bir_lowering=False, debug=False)

    xT_d = nc.dram_tensor("xT", [H, nkv], f32, kind="ExternalInput").ap()
    xqT_d = nc.dram_tensor("xqT", [H, nq], f32, kind="ExternalInput").ap()
    wqT_d = nc.dram_tensor("wqT", [H, H], f32, kind="ExternalInput").ap()
    wkT_d = nc.dram_tensor("wkT", [H, H], f32, kind="ExternalInput").ap()
    wvT_d = nc.dram_tensor("wvT", [H, H], f32, kind="ExternalInput").ap()
    bq_d = nc.dram_tensor("bq", [H, 1], f32, kind="ExternalInput").ap()
    bk_d = nc.dram_tensor("bk", [H, 1], f32, kind="ExternalInput").ap()
    att_d = nc.dram_tensor("att", [nq, H], f32, kind="ExternalOutput").ap()

    with tile.TileContext(nc) as tc:
        with tc.tile_pool(name="io", bufs=1) as io, \
             tc.tile_pool(name="kqv", bufs=1) as kqv, \
             tc.tile_pool(name="expp", bufs=6) as expp, \
             tc.tile_pool(name="op", bufs=4) as op, \
             tc.tile_pool(name="psmm", bufs=3, space="PSUM") as psmm, \
             tc.tile_pool(name="psatt", bufs=4, space="PSUM") as psatt:

            # ---- input loads (gpsimd DMA casts f32 -> f32r on the fly) ----
            xt = [io.tile([P, nkv], FR, tag=f"xt{h}", name=f"xt{h}") for h in range(2)]
            xq = [io.tile([P, nq], FR, tag=f"xq{h}", name=f"xq{h}") for h in range(2)]
            wq = [io.tile([P, H], FR, tag=f"wq{h}", name=f"wq{h}") for h in range(2)]
            wk = [io.tile([P, H], FR, tag=f"wk{h}", name=f"wk{h}") for h in range(2)]
            wv = [io.tile([P, H], FR, tag=f"wv{h}", name=f"wv{h}") for h in range(2)]
            bqt = [io.tile([P, 1], f32, tag=f"bq{h}", name=f"bq{h}") for h in range(2)]
            bkt = [io.tile([P, 1], f32, tag=f"bk{h}", name=f"bk{h}") for h in range(2)]
            # weights/biases first (small), then x in 1024-column chunks so
            # the projection matmuls can start before the full x has landed
            # (Tile tracks deps per dma_start instruction).
            for h in range(2):
                hs = slice(h * P, (h + 1) * P)
                nc.gpsimd.dma_start(wq[h][:], wqT_d[hs, :])
                nc.gpsimd.dma_start(wk[h][:], wkT_d[hs, :])
                nc.gpsimd.dma_start(wv[h][:], wvT_d[hs, :])
                nc.sync.dma_start(bqt[h][:], bq_d[hs, :])
                nc.sync.dma_start(bkt[h][:], bk_d[hs, :])
            xchunk = 1024
            for c0 in range(0, nkv, xchunk):
                cs = slice(c0, min(c0 + xchunk, nkv))
                for h in range(2):
                    hs = slice(h * P, (h + 1) * P)
                    nc.gpsimd.dma_start(xt[h][:, cs], xT_d[hs, cs])
            for c0 in range(0, nq, xchunk):
                cs = slice(c0, min(c0 + xchunk, nq))
                for h in range(2):
                    hs = slice(h * P, (h + 1) * P)
                    nc.gpsimd.dma_start(xq[h][:, cs], xqT_d[hs, cs])

            # ---- projections ----
            # K_T[o, j] = sum_h WkT[h, o] * xT[h, j]   (+ bk[o])
            kT = [kqv.tile([P, nkv], FR, tag=f"kT{oc}", name=f"kT{oc}") for oc in range(2)]
            qT = [kqv.tile([P, nq], FR, tag=f"qT{oc}", name=f"qT{oc}") for oc in range(2)]
            for oc in range(2):
                ocs = slice(oc * P, (oc + 1) * P)
                for s in range(nkv // kseg):
                    ss = slice(s * kseg, (s + 1) * kseg)
                    pk = psmm.tile([P, kseg], f32, tag="mm", name="pk")
                    for h in range(2):
                        nc.tensor.matmul(pk[:], wk[h][:, ocs], xt[h][:, ss],
                                         start=(h == 0), stop=(h == 1))
                    nc.vector.tensor_scalar_add(kT[oc][:, ss], pk[:], bkt[oc][:])
                for s in range(nq // qseg):
                    ss = slice(s * qseg, (s + 1) * qseg)
                    pq = psmm.tile([P, qseg], f32, tag="mm", name="pq")
                    for h in range(2):
                        nc.tensor.matmul(pq[:], wq[h][:, ocs], xq[h][:, ss],
                                         start=(h == 0), stop=(h == 1))
                    nc.vector.tensor_scalar_add(qT[oc][:, ss], pq[:], bqt[oc][:])

            # V[n, o] = sum_h xT[h, n] * WvT[h, o]  (no bias: added on host),
            # plus two trailing columns of ones: column H yields the softmax
            # denominator off the AV matmul; two of them because fp32r
            # matmuls require an even moving free-dim count.
            ones2 = io.tile([P, 2], f32, tag="ones2", name="ones2")
            nc.vector.memset(ones2[:], 1.0)
            vt = [kqv.tile([P, H + 2], AVT, tag=f"v{j}", name=f"v{j}") for j in range(JC)]
            for j in range(JC):
                js = slice(j * P, (j + 1) * P)
                pv = psmm.tile([P, H], f32, tag="mm", name="pv")
                for h in range(2):
                    nc.tensor.matmul(pv[:], xt[h][:, js], wv[h][:],
                                     start=(h == 0), stop=(h == 1))
                nc.vector.tensor_copy(vt[j][:, 0:H], pv[:])
                nc.vector.tensor_copy(vt[j][:, H:H + 2], ones2[:])

            # ---- attention blocks ----
            for blk in range(NBLK):
                bs = slice(blk * iblk, (blk + 1) * iblk)
                att_ps = [psatt.tile([P, H + 2], f32, tag="att", name="attps")
                          for _ in range(ICH)]
                for jc in range(JC):
                    jcs = slice(jc * P, (jc + 1) * P)
                    sc = psmm.tile([P, iblk], f32, tag="mm", name="sc")
                    for oc in range(2):
                        nc.tensor.matmul(sc[:], kT[oc][:, jcs], qT[oc][:, bs],
                                         start=(oc == 0), stop=(oc == 1))
                    ex = expp.tile([P, iblk], AVT, tag="ex", name="ex")
                    nc.scalar.activation(ex[:], sc[:], Exp)
                    for ic in range(ICH):
                        ics = slice(ic * P, (ic + 1) * P)
                        nc.tensor.matmul(att_ps[ic][:], ex[:, ics], vt[jc][:],
                                         start=(jc == 0), stop=(jc == JC - 1))
                for ic in range(ICH):
                    rec = op.tile([P, 1], f32, tag="rec", name="rec")
                    nc.vector.reciprocal(rec[:], att_ps[ic][:, H:H + 1])
                    ao = op.tile([P, H], f32, tag="ao", name="ao")
                    nc.vector.tensor_scalar_mul(ao[:], att_ps[ic][:, 0:H], rec[:])
                    r0 = blk * iblk + ic * P
                    nc.sync.dma_start(att_d[r0:r0 + P, :], ao[:])

    nc.compile()
    return nc


_NC_CACHE = {}


def _get_nc(nkv=N, nq=NQ, iblk=512):
    key = (nkv, nq, iblk)
    if key not in _NC_CACHE:
        _NC_CACHE[key] = build_nc(*key)
    return _NC_CACHE[key]


def _make_in_maps(x, Wq, bq, Wk, bk, Wv):
    wqT = np.ascontiguousarray(Wq.T)
    wkT = np.ascontiguousarray(Wk.T)
    wvT = np.ascontiguousarray(Wv.T)
    bq2 = np.ascontiguousarray(bq.reshape(H, 1))
    bk2 = np.ascontiguousarray(bk.reshape(H, 1))
    xT = [np.ascontiguousarray(x[b].T) for b in range(B)]
    in_maps = []
    for c in range(8):
        b, half = c // 2, c % 2
        in_maps.append({
            "xT": xT[b],
            "xqT": np.ascontiguousarray(x[b, half * NQ:(half + 1) * NQ, :].T),
            "wqT": wqT, "wkT": wkT, "wvT": wvT,
            "bq": bq2, "bk": bk2,
        })
    return in_maps


def _run(inputs, trace=False):
    from concourse.bass_utils import run_bass_kernel_spmd

    x = np.asarray(inputs["x"], dtype=np.float32)
    Wq = np.asarray(inputs["Wq"], dtype=np.float32)
    bq = np.asarray(inputs["bq"], dtype=np.float32)
    Wk = np.asarray(inputs["Wk"], dtype=np.float32)
    bk = np.asarray(inputs["bk"], dtype=np.float32)
    Wv = np.asarray(inputs["Wv"], dtype=np.float32)
    bv = np.asarray(inputs["bv"], dtype=np.float32)

    nc = _get_nc()
    in_maps = _make_in_maps(x, Wq, bq, Wk, bk, Wv)
    res = run_bass_kernel_spmd(nc, in_maps, list(range(8)), trace=trace)

    out = np.empty((B, N, H), dtype=np.float32)
    for c in range(8):
        b, half = c // 2, c % 2
        out[b, half * NQ:(half + 1) * NQ, :] = res.results[c]["att"] + bv
    return out, res


def kernel(**inputs) -> np.ndarray:
    out, _ = _run(inputs, trace=False)
    return out


# revision 9
# speedup vs baseline: 1.0313x; 1.0011x over previous
"""Self-attention (CrossAttention module with q=k=v=x) kernel for Trainium2.

Problem: x [B=4, N=4096, H=256] fp32; Wq/Wk/Wv [256,256], bq/bk/bv [256].
  q = x@Wq.T+bq ; k = x@Wk.T+bk ; v = x@Wv.T+bv
  out = softmax(q@k.T) @ v          (no 1/sqrt(d) scaling)

Sharding: 8 cores = batch (4) x query-halves (2). Each core holds the full
K/V sequence for its batch element and 2048 query rows.

Per-core layout strategy (all matmuls in float32r = TF32 on the PE, fp32
accumulation):
  - Host pre-transposes x -> xT [H, N] and weights -> W.T [h, o] so that
    Q^T/K^T land in [o, n] layout directly off the projection matmuls.
  - scores are computed TRANSPOSED: S_T[j, i] = sum_o K_T[o,j] * Q_T[o,i]
    (K_T chunk stationary, Q_T moving), so exp(S_T) feeds the AV matmul
    as the stationary operand with no on-chip transpose.
  - V is produced in natural [n, o] layout with a 257th column of ones:
    att_psum[i, 0:256] = sum_j P_T[j,i] V[j,:], att_psum[i, 256] = row sum
    of P (the softmax denominator) -- the denominator comes free.
  - normalize with DVE reciprocal + per-partition broadcast multiply.
  - v-bias is added on the host after gathering (softmax rows sum to 1).
"""

import sys

import numpy as np

if "/opt/trn_rl_repo" not in sys.path:
    sys.path.insert(0, "/opt/trn_rl_repo")

B, N, H = 4, 4096, 256
P = 128
NQ = N // 2  # query rows per core


def build_nc(nkv=N, nq=NQ, iblk=512):
    import concourse.mybir as mybir
    import concourse.tile as tile
    from concourse import bacc

    f32 = mybir.dt.float32
    FR = mybir.dt.float16   # projections + scores matmul dtype (11-bit mantissa)
    AVT = mybir.dt.bfloat16  # exp(S) and V dtype: needs fp32-like range
    Exp = mybir.ActivationFunctionType.Exp

    assert nkv % 512 == 0 and nq % iblk == 0 and iblk % P == 0 and iblk <= 512
    JC = nkv // P          # key chunks
    ICH = iblk // P        # query sub-chunks per block
    NBLK = nq // iblk      # query blocks
    kseg = 512             # K_T projection segment (moving free dim)
    qseg = min(512, nq)

    nc = bacc.Bacc("TRN2", target_bir_lowering=False, debug=False)

    xT_d = nc.dram_tensor("xT", [H, nkv], FR, kind="ExternalInput").ap()
    xqT_d = nc.dram_tensor("xqT", [H, nq], FR, kind="ExternalInput").ap()
    wqT_d = nc.dram_tensor("wqT", [H, H], FR, kind="ExternalInput").ap()
    wkT_d = nc.dram_tensor("wkT", [H, H], FR, kind="ExternalInput").ap()
    wvT_d = nc.dram_tensor("wvT", [H, H], FR, kind="ExternalInput").ap()
    bq_d = nc.dram_tensor("bq", [H, 1], f32, kind="ExternalInput").ap()
    bk_d = nc.dram_tensor("bk", [H, 1], f32, kind="ExternalInput").ap()
    att_d = nc.dram_tensor("att", [nq, H], f32, kind="ExternalOutput").ap()

    with tile.TileContext(nc) as tc:
        with tc.tile_pool(name="io", bufs=1) as io, \
             tc.tile_pool(name="kqv", bufs=1) as kqv, \
             tc.tile_pool(name="expp", bufs=6) as expp, \
             tc.tile_pool(name="op", bufs=4) as op, \
             tc.tile_pool(name="psmm", bufs=3, space="PSUM") as psmm, \
             tc.tile_pool(name="psatt", bufs=4, space="PSUM") as psatt:

            # ---- input loads (gpsimd DMA casts f32 -> f32r on the fly) ----
            xt = [io.tile([P, nkv], FR, tag=f"xt{h}", name=f"xt{h}") for h in range(2)]
            xq = [io.tile([P, nq], FR, tag=f"xq{h}", name=f"xq{h}") for h in range(2)]
            wq = [io.tile([P, H], FR, tag=f"wq{h}", name=f"wq{h}") for h in range(2)]
            wk = [io.tile([P, H], FR, tag=f"wk{h}", name=f"wk{h}") for h in range(2)]
            wv = [io.tile([P, H], FR, tag=f"wv{h}", name=f"wv{h}") for h in range(2)]
            bqt = [io.tile([P, 1], f32, tag=f"bq{h}", name=f"bq{h}") for h in range(2)]
            bkt = [io.tile([P, 1], f32, tag=f"bk{h}", name=f"bk{h}") for h in range(2)]
            # weights/biases first (small), then x in 1024-column chunks so
            # the projection matmuls can start before the full x has landed
            # (Tile tracks deps per dma_start instruction).
            for h in range(2):
                hs = slice(h * P, (h + 1) * P)
                nc.sync.dma_start(wq[h][:], wqT_d[hs, :])
                nc.sync.dma_start(wk[h][:], wkT_d[hs, :])
                nc.sync.dma_start(wv[h][:], wvT_d[hs, :])
                nc.sync.dma_start(bqt[h][:], bq_d[hs, :])
                nc.sync.dma_start(bkt[h][:], bk_d[hs, :])
            xchunk = 1024
            for c0 in range(0, nq, xchunk):
                cs = slice(c0, min(c0 + xchunk, nq))
                for h in range(2):
                    hs = slice(h * P, (h + 1) * P)
                    nc.sync.dma_start(xq[h][:, cs], xqT_d[hs, cs])
            for c0 in range(0, nkv, xchunk):
                cs = slice(c0, min(c0 + xchunk, nkv))
                for h in range(2):
                    hs = slice(h * P, (h + 1) * P)
                    nc.sync.dma_start(xt[h][:, cs], xT_d[hs, cs])

            # ---- projections ----
            # K_T[o, j] = sum_h WkT[h, o] * xT[h, j]   (+ bk[o])
            kT = [kqv.tile([P, nkv], FR, tag=f"kT{oc}", name=f"kT{oc}") for oc in range(2)]
            qT = [kqv.tile([P, nq], FR, tag=f"qT{oc}", name=f"qT{oc}") for oc in range(2)]
            for oc in range(2):
                ocs = slice(oc * P, (oc + 1) * P)
                for s in range(nkv // kseg):
                    ss = slice(s * kseg, (s + 1) * kseg)
                    pk = psmm.tile([P, kseg], f32, tag="mm", name="pk")
                    for h in range(2):
                        nc.tensor.matmul(pk[:], wk[h][:, ocs], xt[h][:, ss],
                                         start=(h == 0), stop=(h == 1))
                    nc.vector.tensor_scalar_add(kT[oc][:, ss], pk[:], bkt[oc][:])
                for s in range(nq // qseg):
                    ss = slice(s * qseg, (s + 1) * qseg)
                    pq = psmm.tile([P, qseg], f32, tag="mm", name="pq")
                    for h in range(2):
                        nc.tensor.matmul(pq[:], wq[h][:, ocs], xq[h][:, ss],
                                         start=(h == 0), stop=(h == 1))
                    nc.vector.tensor_scalar_add(qT[oc][:, ss], pq[:], bqt[oc][:])

            # V[n, o] = sum_h xT[h, n] * WvT[h, o]  (no bias: added on host),
            # plus two trailing columns of ones: column H yields the softmax
            # denominator off the AV matmul; two of them because fp32r
            # matmuls require an even moving free-dim count.
            ones2 = io.tile([P, 2], f32, tag="ones2", name="ones2")
            nc.vector.memset(ones2[:], 1.0)
            vt = [kqv.tile([P, H + 2], AVT, tag=f"v{j}", name=f"v{j}") for j in range(JC)]
            for j in range(JC):
                js = slice(j * P, (j + 1) * P)
                pv = psmm.tile([P, H], f32, tag="mm", name="pv")
                for h in range(2):
                    nc.tensor.matmul(pv[:], xt[h][:, js], wv[h][:],
                                     start=(h == 0), stop=(h == 1))
                nc.vector.tensor_copy(vt[j][:, 0:H], pv[:])
                nc.vector.tensor_copy(vt[j][:, H:H + 2], ones2[:])

            # ---- attention blocks ----
            for blk in range(NBLK):
                bs = slice(blk * iblk, (blk + 1) * iblk)
                att_ps = [psatt.tile([P, H + 2], f32, tag="att", name="attps")
                          for _ in range(ICH)]
                for jc in range(JC):
                    jcs = slice(jc * P, (jc + 1) * P)
                    sc = psmm.tile([P, iblk], f32, tag="mm", name="sc")
                    for oc in range(2):
                        nc.tensor.matmul(sc[:], kT[oc][:, jcs], qT[oc][:, bs],
                                         start=(oc == 0), stop=(oc == 1))
                    ex = expp.tile([P, iblk], AVT, tag="ex", name="ex")
                    nc.scalar.activation(ex[:], sc[:], Exp)
                    for ic in range(ICH):
                        ics = slice(ic * P, (ic + 1) * P)
                        nc.tensor.matmul(att_ps[ic][:], ex[:, ics], vt[jc][:],
                                         start=(jc == 0), stop=(jc == JC - 1))
                for ic in range(ICH):
                    rec = op.tile([P, 1], f32, tag="rec", name="rec")
                    nc.vector.reciprocal(rec[:], att_ps[ic][:, H:H + 1])
                    ao = op.tile([P, H], f32, tag="ao", name="ao")
                    nc.vector.tensor_scalar_mul(ao[:], att_ps[ic][:, 0:H], rec[:])
                    r0 = blk * iblk + ic * P
                    nc.sync.dma_start(att_d[r0:r0 + P, :], ao[:])

    nc.compile()
    return nc


_NC_CACHE = {}


def _get_nc(nkv=N, nq=NQ, iblk=512):
    key = (nkv, nq, iblk)
    if key not in _NC_CACHE:
        _NC_CACHE[key] = build_nc(*key)
    return _NC_CACHE[key]


def _make_in_maps(x, Wq, bq, Wk, bk, Wv):
    wqT = np.ascontiguousarray(Wq.T.astype(np.float16))
    wkT = np.ascontiguousarray(Wk.T.astype(np.float16))
    wvT = np.ascontiguousarray(Wv.T.astype(np.float16))
    bq2 = np.ascontiguousarray(bq.reshape(H, 1))
    bk2 = np.ascontiguousarray(bk.reshape(H, 1))
    x16 = x.astype(np.float16)
    xT = [np.ascontiguousarray(x16[b].T) for b in range(B)]
    in_maps = []
    for c in range(8):
        b, half = c // 2, c % 2
        in_maps.append({
            "xT": xT[b],
            "xqT": np.ascontiguousarray(x16[b, half * NQ:(half + 1) * NQ, :].T),
            "wqT": wqT, "wkT": wkT, "wvT": wvT,
            "bq": bq2, "bk": bk2,
        })
    return in_maps


def _run(inputs, trace=False):
    from concourse.bass_utils import run_bass_kernel_spmd

    x = np.asarray(inputs["x"], dtype=np.float32)
    Wq = np.asarray(inputs["Wq"], dtype=np.float32)
    bq = np.asarray(inputs["bq"], dtype=np.float32)
    Wk = np.asarray(inputs["Wk"], dtype=np.float32)
    bk = np.asarray(inputs["bk"], dtype=np.float32)
    Wv = np.asarray(inputs["Wv"], dtype=np.float32)
    bv = np.asarray(inputs["bv"], dtype=np.float32)

    nc = _get_nc()
    in_maps = _make_in_maps(x, Wq, bq, Wk, bk, Wv)
    res = run_bass_kernel_spmd(nc, in_maps, list(range(8)), trace=trace)

    out = np.empty((B, N, H), dtype=np.float32)
    for c in range(8):
        b, half = c // 2, c % 2
        out[b, half * NQ:(half + 1) * NQ, :] = res.results[c]["att"] + bv
    return out, res


def kernel(**inputs) -> np.ndarray:
    out, _ = _run(inputs, trace=False)
    return out


# revision 12
# speedup vs baseline: 1.1046x; 1.0711x over previous
"""Self-attention (CrossAttention module with q=k=v=x) kernel for Trainium2.

Problem: x [B=4, N=4096, H=256] fp32; Wq/Wk/Wv [256,256], bq/bk/bv [256].
  q = x@Wq.T+bq ; k = x@Wk.T+bk ; v = x@Wv.T+bv
  out = softmax(q@k.T) @ v          (no 1/sqrt(d) scaling)

Sharding: 8 cores = batch (4) x query-halves (2). Each core holds the full
K/V sequence for its batch element and 2048 query rows.

Per-core layout strategy (all matmuls in float32r = TF32 on the PE, fp32
accumulation):
  - Host pre-transposes x -> xT [H, N] and weights -> W.T [h, o] so that
    Q^T/K^T land in [o, n] layout directly off the projection matmuls.
  - scores are computed TRANSPOSED: S_T[j, i] = sum_o K_T[o,j] * Q_T[o,i]
    (K_T chunk stationary, Q_T moving), so exp(S_T) feeds the AV matmul
    as the stationary operand with no on-chip transpose.
  - V is produced in natural [n, o] layout with a 257th column of ones:
    att_psum[i, 0:256] = sum_j P_T[j,i] V[j,:], att_psum[i, 256] = row sum
    of P (the softmax denominator) -- the denominator comes free.
  - normalize with DVE reciprocal + per-partition broadcast multiply.
  - v-bias is added on the host after gathering (softmax rows sum to 1).
"""

import sys

import numpy as np

if "/opt/trn_rl_repo" not in sys.path:
    sys.path.insert(0, "/opt/trn_rl_repo")

B, N, H = 4, 4096, 256
P = 128
NQ = N // 2  # query rows per core


def build_nc(nkv=N, nq=NQ, iblk=512):
    import concourse.mybir as mybir
    import concourse.tile as tile
    from concourse import bacc

    f32 = mybir.dt.float32
    FR = mybir.dt.float16   # projections + scores matmul dtype (11-bit mantissa)
    AVT = mybir.dt.bfloat16  # exp(S) and V dtype: needs fp32-like range
    Exp = mybir.ActivationFunctionType.Exp

    assert nkv % 512 == 0 and nq % iblk == 0 and iblk % P == 0 and iblk <= 512
    JC = nkv // P          # key chunks
    ICH = iblk // P        # query sub-chunks per block
    NBLK = nq // iblk      # query blocks
    kseg = 512             # K_T projection segment (moving free dim)
    qseg = min(512, nq)

    nc = bacc.Bacc("TRN2", target_bir_lowering=False, debug=False)

    xT_d = nc.dram_tensor("xT", [H, nkv], FR, kind="ExternalInput").ap()
    xqT_d = nc.dram_tensor("xqT", [H, nq], FR, kind="ExternalInput").ap()
    wqT_d = nc.dram_tensor("wqT", [H, H], FR, kind="ExternalInput").ap()
    wkT_d = nc.dram_tensor("wkT", [H, H], FR, kind="ExternalInput").ap()
    wvT_d = nc.dram_tensor("wvT", [H, H], FR, kind="ExternalInput").ap()
    bq_d = nc.dram_tensor("bq", [H, 1], f32, kind="ExternalInput").ap()
    bk_d = nc.dram_tensor("bk", [H, 1], f32, kind="ExternalInput").ap()
    att_d = nc.dram_tensor("att", [nq, H], f32, kind="ExternalOutput").ap()
    warm_d = nc.dram_tensor("warm", [P, 2], f32, kind="ExternalOutput").ap()

    with tile.TileContext(nc) as tc:
        with tc.tile_pool(name="io", bufs=1) as io, \
             tc.tile_pool(name="kqv", bufs=1) as kqv, \
             tc.tile_pool(name="expp", bufs=JC + 8) as expp, \
             tc.tile_pool(name="op", bufs=4) as op, \
             tc.tile_pool(name="psmm", bufs=3, space="PSUM") as psmm, \
             tc.tile_pool(name="psatt", bufs=5, space="PSUM") as psatt:

            # ---- input loads (gpsimd DMA casts f32 -> f32r on the fly) ----
            xt = [io.tile([P, nkv], FR, tag=f"xt{h}", name=f"xt{h}") for h in range(2)]
            xq = [io.tile([P, nq], FR, tag=f"xq{h}", name=f"xq{h}") for h in range(2)]
            wq = [io.tile([P, H], FR, tag=f"wq{h}", name=f"wq{h}") for h in range(2)]
            wk = [io.tile([P, H], FR, tag=f"wk{h}", name=f"wk{h}") for h in range(2)]
            wv = [io.tile([P, H], FR, tag=f"wv{h}", name=f"wv{h}") for h in range(2)]
            bqt = [io.tile([P, 1], f32, tag=f"bq{h}", name=f"bq{h}") for h in range(2)]
            bkt = [io.tile([P, 1], f32, tag=f"bk{h}", name=f"bk{h}") for h in range(2)]
            # weights/biases first (small), then x in 1024-column chunks so
            # the projection matmuls can start before the full x has landed
            # (Tile tracks deps per dma_start instruction).
            for h in range(2):
                hs = slice(h * P, (h + 1) * P)
                nc.sync.dma_start(wq[h][:], wqT_d[hs, :])
                nc.sync.dma_start(wk[h][:], wkT_d[hs, :])
                nc.sync.dma_start(wv[h][:], wvT_d[hs, :])
                nc.sync.dma_start(bqt[h][:], bq_d[hs, :])
                nc.sync.dma_start(bkt[h][:], bk_d[hs, :])
            xchunk = 1024
            for c0 in range(0, nq, xchunk):
                cs = slice(c0, min(c0 + xchunk, nq))
                for h in range(2):
                    hs = slice(h * P, (h + 1) * P)
                    nc.sync.dma_start(xq[h][:, cs], xqT_d[hs, cs])
            for c0 in range(0, nkv, xchunk):
                cs = slice(c0, min(c0 + xchunk, nkv))
                for h in range(2):
                    hs = slice(h * P, (h + 1) * P)
                    nc.sync.dma_start(xt[h][:, cs], xT_d[hs, cs])

            # ---- PE warm-up ----
            # ~10 junk matmuls on the (tiny, early-arriving) weight tiles
            # keep the PE busy while x streams in, so the HAM clock gate
            # reaches 2.4 GHz before the real work starts.  The result is
            # written to a throwaway output so DCE keeps the chain.
            wps = psmm.tile([P, H], f32, tag="mm", name="wps")
            for r in range(10):
                nc.tensor.matmul(wps[:], wq[0][:, 0:P], wq[1][:],
                                 start=(r == 0), stop=(r == 9))
            wsb = op.tile([P, 2], f32, tag="wsb", name="wsb")
            nc.vector.tensor_copy(wsb[:], wps[:, 0:2])
            nc.sync.dma_start(warm_d[:], wsb[:])

            # ---- projections ----
            # Q_T first (xq arrives first), then K_T / V interleaved in
            # xT-chunk arrival order.
            kT = [kqv.tile([P, nkv], FR, tag=f"kT{oc}", name=f"kT{oc}") for oc in range(2)]
            qT = [kqv.tile([P, nq], FR, tag=f"qT{oc}", name=f"qT{oc}") for oc in range(2)]
            ones2 = io.tile([P, 2], f32, tag="ones2", name="ones2")
            nc.vector.memset(ones2[:], 1.0)
            vt = [kqv.tile([P, H + 2], AVT, tag=f"v{j}", name=f"v{j}") for j in range(JC)]

            for s in range(nq // qseg):
                ss = slice(s * qseg, (s + 1) * qseg)
                for oc in range(2):
                    ocs = slice(oc * P, (oc + 1) * P)
                    pq = psmm.tile([P, qseg], f32, tag="mm", name="pq")
                    for h in range(2):
                        nc.tensor.matmul(pq[:], wq[h][:, ocs], xq[h][:, ss],
                                         start=(h == 0), stop=(h == 1))
                    nc.vector.tensor_scalar_add(qT[oc][:, ss], pq[:], bqt[oc][:])

            # K_T[o, j] = sum_h WkT[h, o] * xT[h, j]   (+ bk[o]);
            # V[n, o] = sum_h xT[h, n] * WvT[h, o] (no bias: added on host),
            # plus two trailing columns of ones: column H yields the softmax
            # denominator straight off the AV matmul.
            for c in range(0, nkv, xchunk):
                for s in range(c // kseg, min(nkv, c + xchunk) // kseg):
                    ss = slice(s * kseg, (s + 1) * kseg)
                    for oc in range(2):
                        ocs = slice(oc * P, (oc + 1) * P)
                        pk = psmm.tile([P, kseg], f32, tag="mm", name="pk")
                        for h in range(2):
                            nc.tensor.matmul(pk[:], wk[h][:, ocs], xt[h][:, ss],
                                             start=(h == 0), stop=(h == 1))
                        nc.vector.tensor_scalar_add(kT[oc][:, ss], pk[:], bkt[oc][:])
                for j in range(c // P, min(nkv, c + xchunk) // P):
                    js = slice(j * P, (j + 1) * P)
                    pv = psmm.tile([P, H], f32, tag="mm", name="pv")
                    for h in range(2):
                        nc.tensor.matmul(pv[:], xt[h][:, js], wv[h][:],
                                         start=(h == 0), stop=(h == 1))
                    nc.vector.tensor_copy(vt[j][:, 0:H], pv[:])
                    nc.vector.tensor_copy(vt[j][:, H:H + 2], ones2[:])

            # ---- attention blocks ----
            # Blocks are processed in pairs: the scores matmuls for both
            # blocks of a pair share each kT stationary load (halving the
            # scores LDWEIGHTS count, which is serialized with the matmuls
            # on the PE).  Block b0's AV runs inline per key-chunk; block
            # b1's exp(S) tiles are buffered in SBUF and consumed in a
            # second AV sweep (PSUM can only hold one block's accumulators
            # plus the rotating scores tiles).
            def av_sweep(att_ps, exs, blk):
                for jc in range(JC):
                    for ic in range(ICH):
                        ics = slice(ic * P, (ic + 1) * P)
                        nc.tensor.matmul(att_ps[ic][:], exs[jc][:, ics],
                                         vt[jc][:],
                                         start=(jc == 0), stop=(jc == JC - 1))

            def normalize(att_ps, blk):
                for ic in range(ICH):
                    rec = op.tile([P, 1], f32, tag="rec", name="rec")
                    nc.vector.reciprocal(rec[:], att_ps[ic][:, H:H + 1])
                    ao = op.tile([P, H], f32, tag="ao", name="ao")
                    nc.vector.tensor_scalar_mul(ao[:], att_ps[ic][:, 0:H], rec[:])
                    r0 = blk * iblk + ic * P
                    nc.sync.dma_start(att_d[r0:r0 + P, :], ao[:])

            blk = 0
            while blk < NBLK:
                paired = blk + 1 < NBLK
                bs0 = slice(blk * iblk, (blk + 1) * iblk)
                bs1 = slice((blk + 1) * iblk, (blk + 2) * iblk)
                att_ps = [psatt.tile([P, H + 2], f32, tag="att", name="attps")
                          for _ in range(ICH)]
                exs1 = []
                for jc in range(JC):
                    jcs = slice(jc * P, (jc + 1) * P)
                    sc0 = psmm.tile([P, iblk], f32, tag="mm", name="sc0")
                    sc1 = psmm.tile([P, iblk], f32, tag="mm", name="sc1") if paired else None
                    for oc in range(2):
                        nc.tensor.matmul(sc0[:], kT[oc][:, jcs], qT[oc][:, bs0],
                                         start=(oc == 0), stop=(oc == 1))
                        if paired:
                            nc.tensor.matmul(sc1[:], kT[oc][:, jcs], qT[oc][:, bs1],
                                             start=(oc == 0), stop=(oc == 1))
                    ex0 = expp.tile([P, iblk], AVT, tag="ex", name="ex0")
                    nc.scalar.activation(ex0[:], sc0[:], Exp)
                    if paired:
                        ex1 = expp.tile([P, iblk], AVT, tag="ex", name="ex1")
                        nc.scalar.activation(ex1[:], sc1[:], Exp)
                        exs1.append(ex1)
                    for ic in range(ICH):
                        ics = slice(ic * P, (ic + 1) * P)
                        nc.tensor.matmul(att_ps[ic][:], ex0[:, ics], vt[jc][:],
                                         start=(jc == 0), stop=(jc == JC - 1))
                normalize(att_ps, blk)
                if paired:
                    att_ps1 = [psatt.tile([P, H + 2], f32, tag="att", name="attps1")
                               for _ in range(ICH)]
                    av_sweep(att_ps1, exs1, blk + 1)
                    normalize(att_ps1, blk + 1)
                blk += 2 if paired else 1

    nc.compile()
    return nc


_NC_CACHE = {}


def _get_nc(nkv=N, nq=NQ, iblk=512):
    key = (nkv, nq, iblk)
    if key not in _NC_CACHE:
        _NC_CACHE[key] = build_nc(*key)
    return _NC_CACHE[key]


def _make_in_maps(x, Wq, bq, Wk, bk, Wv):
    wqT = np.ascontiguousarray(Wq.T.astype(np.float16))
    wkT = np.ascontiguousarray(Wk.T.astype(np.float16))
    wvT = np.ascontiguousarray(Wv.T.astype(np.float16))
    bq2 = np.ascontiguousarray(bq.reshape(H, 1))
    bk2 = np.ascontiguousarray(bk.reshape(H, 1))
    x16 = x.astype(np.float16)
    xT = [np.ascontiguousarray(x16[b].T) for b in range(B)]
    in_maps = []
    for c in range(8):
        b, half = c // 2, c % 2
        in_maps.append({
            "xT": xT[b],
            "xqT": np.ascontiguousarray(x16[b, half * NQ:(half + 1) * NQ, :].T),
            "wqT": wqT, "wkT": wkT, "wvT": wvT,
            "bq": bq2, "bk": bk2,
        })
    return in_maps


def _run(inputs, trace=False):
    from concourse.bass_utils import run_bass_kernel_spmd

    x = np.asarray(inputs["x"], dtype=np.float32)
    Wq = np.asarray(inputs["Wq"], dtype=np.float32)
    bq = np.asarray(inputs["bq"], dtype=np.float32)
    Wk = np.asarray(inputs["Wk"], dtype=np.float32)
    bk = np.asarray(inputs["bk"], dtype=np.float32)
    Wv = np.asarray(inputs["Wv"], dtype=np.float32)
    bv = np.asarray(inputs["bv"], dtype=np.float32)

    nc = _get_nc()
    in_maps = _make_in_maps(x, Wq, bq, Wk, bk, Wv)
    res = run_bass_kernel_spmd(nc, in_maps, list(range(8)), trace=trace)

    out = np.empty((B, N, H), dtype=np.float32)
    for c in range(8):
        b, half = c // 2, c % 2
        out[b, half * NQ:(half + 1) * NQ, :] = res.results[c]["att"] + bv
    return out, res


def kernel(**inputs) -> np.ndarray:
    out, _ = _run(inputs, trace=False)
    return out


# revision 15
# speedup vs baseline: 1.1072x; 1.0023x over previous
"""Self-attention (CrossAttention module with q=k=v=x) kernel for Trainium2.

Problem: x [B=4, N=4096, H=256] fp32; Wq/Wk/Wv [256,256], bq/bk/bv [256].
  q = x@Wq.T+bq ; k = x@Wk.T+bk ; v = x@Wv.T+bv
  out = softmax(q@k.T) @ v          (no 1/sqrt(d) scaling)

Sharding: 8 cores = batch (4) x query-halves (2). Each core holds the full
K/V sequence for its batch element and 2048 query rows.

Per-core layout strategy (all matmuls in float32r = TF32 on the PE, fp32
accumulation):
  - Host pre-transposes x -> xT [H, N] and weights -> W.T [h, o] so that
    Q^T/K^T land in [o, n] layout directly off the projection matmuls.
  - scores are computed TRANSPOSED: S_T[j, i] = sum_o K_T[o,j] * Q_T[o,i]
    (K_T chunk stationary, Q_T moving), so exp(S_T) feeds the AV matmul
    as the stationary operand with no on-chip transpose.
  - V is produced in natural [n, o] layout with a 257th column of ones:
    att_psum[i, 0:256] = sum_j P_T[j,i] V[j,:], att_psum[i, 256] = row sum
    of P (the softmax denominator) -- the denominator comes free.
  - normalize with DVE reciprocal + per-partition broadcast multiply.
  - v-bias is added on the host after gathering (softmax rows sum to 1).
"""

import sys

import numpy as np

if "/opt/trn_rl_repo" not in sys.path:
    sys.path.insert(0, "/opt/trn_rl_repo")

B, N, H = 4, 4096, 256
P = 128
NQ = N // 2  # query rows per core


def build_nc(nkv=N, nq=NQ, iblk=512, salt=0):
    import concourse.mybir as mybir
    import concourse.tile as tile
    from concourse import bacc

    f32 = mybir.dt.float32
    FR = mybir.dt.float16   # projections + scores matmul dtype (11-bit mantissa)
    AVT = mybir.dt.bfloat16  # exp(S) and V dtype: needs fp32-like range
    Exp = mybir.ActivationFunctionType.Exp

    assert nkv % 512 == 0 and nq % iblk == 0 and iblk % P == 0 and iblk <= 512
    JC = nkv // P          # key chunks
    ICH = iblk // P        # query sub-chunks per block
    NBLK = nq // iblk      # query blocks
    kseg = 512             # K_T projection segment (moving free dim)
    qseg = min(512, nq)

    nc = bacc.Bacc("TRN2", target_bir_lowering=False, debug=False)

    xT_d = nc.dram_tensor("xT", [H, nkv], FR, kind="ExternalInput").ap()
    xqT_d = nc.dram_tensor("xqT", [H, nq], FR, kind="ExternalInput").ap()
    wqT_d = nc.dram_tensor("wqT", [H, H], FR, kind="ExternalInput").ap()
    wkT_d = nc.dram_tensor("wkT", [H, H], FR, kind="ExternalInput").ap()
    wvT_d = nc.dram_tensor("wvT", [H, H], FR, kind="ExternalInput").ap()
    bq_d = nc.dram_tensor("bq", [H, 1], f32, kind="ExternalInput").ap()
    bk_d = nc.dram_tensor("bk", [H, 1], f32, kind="ExternalInput").ap()
    att_d = nc.dram_tensor("att", [nq, H], f32, kind="ExternalOutput").ap()
    warm_d = nc.dram_tensor("warm", [P, 2], f32, kind="ExternalOutput").ap()

    with tile.TileContext(nc) as tc:
        with tc.tile_pool(name="io", bufs=1) as io, \
             tc.tile_pool(name="kqv", bufs=1) as kqv, \
             tc.tile_pool(name="expp", bufs=JC + 8) as expp, \
             tc.tile_pool(name="op", bufs=4) as op, \
             tc.tile_pool(name="psmm", bufs=3, space="PSUM") as psmm, \
             tc.tile_pool(name="psatt", bufs=5, space="PSUM") as psatt:

            # ---- input loads (gpsimd DMA casts f32 -> f32r on the fly) ----
            xt = [io.tile([P, nkv], FR, tag=f"xt{h}", name=f"xt{h}") for h in range(2)]
            xq = [io.tile([P, nq], FR, tag=f"xq{h}", name=f"xq{h}") for h in range(2)]
            wq = [io.tile([P, H], FR, tag=f"wq{h}", name=f"wq{h}") for h in range(2)]
            wk = [io.tile([P, H], FR, tag=f"wk{h}", name=f"wk{h}") for h in range(2)]
            wv = [io.tile([P, H], FR, tag=f"wv{h}", name=f"wv{h}") for h in range(2)]
            bqt = [io.tile([P, 1], f32, tag=f"bq{h}", name=f"bq{h}") for h in range(2)]
            bkt = [io.tile([P, 1], f32, tag=f"bk{h}", name=f"bk{h}") for h in range(2)]
            # weights/biases first (small), then x in 1024-column chunks so
            # the projection matmuls can start before the full x has landed
            # (Tile tracks deps per dma_start instruction).
            for h in range(2):
                hs = slice(h * P, (h + 1) * P)
                nc.sync.dma_start(wq[h][:], wqT_d[hs, :])
                nc.sync.dma_start(wk[h][:], wkT_d[hs, :])
                nc.sync.dma_start(wv[h][:], wvT_d[hs, :])
                nc.sync.dma_start(bqt[h][:], bq_d[hs, :])
                nc.sync.dma_start(bkt[h][:], bk_d[hs, :])
            xchunk = 1024
            for c0 in range(0, nq, xchunk):
                cs = slice(c0, min(c0 + xchunk, nq))
                for h in range(2):
                    hs = slice(h * P, (h + 1) * P)
                    nc.sync.dma_start(xq[h][:, cs], xqT_d[hs, cs])
            for c0 in range(0, nkv, xchunk):
                cs = slice(c0, min(c0 + xchunk, nkv))
                for h in range(2):
                    hs = slice(h * P, (h + 1) * P)
                    nc.sync.dma_start(xt[h][:, cs], xT_d[hs, cs])

            # ---- PE warm-up ----
            # ~10 junk matmuls on the (tiny, early-arriving) weight tiles
            # keep the PE busy while x streams in, so the HAM clock gate
            # reaches 2.4 GHz before the real work starts.  The result is
            # written to a throwaway output so DCE keeps the chain.
            wps = psmm.tile([P, H], f32, tag="mm", name="wps")
            nwarm = 24 + salt
            for r in range(nwarm):
                nc.tensor.matmul(wps[:], wq[0][:, 0:P], wq[1][:],
                                 start=(r == 0), stop=(r == nwarm - 1))
            wsb = op.tile([P, 2], f32, tag="wsb", name="wsb")
            nc.vector.tensor_copy(wsb[:], wps[:, 0:2])
            nc.sync.dma_start(warm_d[:], wsb[:])

            # ---- projections ----
            # Q_T first (xq arrives first), then K_T / V interleaved in
            # xT-chunk arrival order.
            kT = [kqv.tile([P, nkv], FR, tag=f"kT{oc}", name=f"kT{oc}") for oc in range(2)]
            qT = [kqv.tile([P, nq], FR, tag=f"qT{oc}", name=f"qT{oc}") for oc in range(2)]
            ones2 = io.tile([P, 2], f32, tag="ones2", name="ones2")
            nc.vector.memset(ones2[:], 1.0)
            vt = [kqv.tile([P, H + 2], AVT, tag=f"v{j}", name=f"v{j}") for j in range(JC)]

            for s in range(nq // qseg):
                ss = slice(s * qseg, (s + 1) * qseg)
                for oc in range(2):
                    ocs = slice(oc * P, (oc + 1) * P)
                    pq = psmm.tile([P, qseg], f32, tag="mm", name="pq")
                    for h in range(2):
                        nc.tensor.matmul(pq[:], wq[h][:, ocs], xq[h][:, ss],
                                         start=(h == 0), stop=(h == 1))
                    nc.vector.tensor_scalar_add(qT[oc][:, ss], pq[:], bqt[oc][:])

            # K_T[o, j] = sum_h WkT[h, o] * xT[h, j]   (+ bk[o]);
            # V[n, o] = sum_h xT[h, n] * WvT[h, o] (no bias: added on host),
            # plus two trailing columns of ones: column H yields the softmax
            # denominator straight off the AV matmul.
            for c in range(0, nkv, xchunk):
                for s in range(c // kseg, min(nkv, c + xchunk) // kseg):
                    ss = slice(s * kseg, (s + 1) * kseg)
                    for oc in range(2):
                        ocs = slice(oc * P, (oc + 1) * P)
                        pk = psmm.tile([P, kseg], f32, tag="mm", name="pk")
                        for h in range(2):
                            nc.tensor.matmul(pk[:], wk[h][:, ocs], xt[h][:, ss],
                                             start=(h == 0), stop=(h == 1))
                        nc.vector.tensor_scalar_add(kT[oc][:, ss], pk[:], bkt[oc][:])
                for j in range(c // P, min(nkv, c + xchunk) // P):
                    js = slice(j * P, (j + 1) * P)
                    pv = psmm.tile([P, H], f32, tag="mm", name="pv")
                    for h in range(2):
                        nc.tensor.matmul(pv[:], xt[h][:, js], wv[h][:],
                                         start=(h == 0), stop=(h == 1))
                    nc.vector.tensor_copy(vt[j][:, 0:H], pv[:])
                    nc.vector.tensor_copy(vt[j][:, H:H + 2], ones2[:])

            # ---- attention blocks ----
            # Blocks are processed in pairs: the scores matmuls for both
            # blocks of a pair share each kT stationary load (halving the
            # scores LDWEIGHTS count, which is serialized with the matmuls
            # on the PE).  Block b0's AV runs inline per key-chunk; block
            # b1's exp(S) tiles are buffered in SBUF and consumed in a
            # second AV sweep (PSUM can only hold one block's accumulators
            # plus the rotating scores tiles).
            def av_sweep(att_ps, exs, blk):
                for jc in range(JC):
                    for ic in range(ICH):
                        ics = slice(ic * P, (ic + 1) * P)
                        nc.tensor.matmul(att_ps[ic][:], exs[jc][:, ics],
                                         vt[jc][:],
                                         start=(jc == 0), stop=(jc == JC - 1))

            def normalize(att_ps, blk):
                for ic in range(ICH):
                    rec = op.tile([P, 1], f32, tag="rec", name="rec")
                    nc.vector.reciprocal(rec[:], att_ps[ic][:, H:H + 1])
                    ao = op.tile([P, H], f32, tag="ao", name="ao")
                    nc.vector.tensor_scalar_mul(ao[:], att_ps[ic][:, 0:H], rec[:])
                    r0 = blk * iblk + ic * P
                    nc.sync.dma_start(att_d[r0:r0 + P, :], ao[:])

            blk = 0
            GRP = 2
            while blk < NBLK:
                g = min(GRP, NBLK - blk)  # blocks in this group
                bss = [slice((blk + b) * iblk, (blk + b + 1) * iblk)
                       for b in range(g)]
                att_ps = [psatt.tile([P, H + 2], f32, tag="att", name="attps")
                          for _ in range(ICH)]
                exs = [[] for _ in range(g)]
                for jc in range(JC):
                    jcs = slice(jc * P, (jc + 1) * P)
                    scs = [psmm.tile([P, iblk], f32, tag="mm", name=f"sc{b}")
                           for b in range(g)]
                    for oc in range(2):
                        for b in range(g):
                            nc.tensor.matmul(scs[b][:], kT[oc][:, jcs],
                                             qT[oc][:, bss[b]],
                                             start=(oc == 0), stop=(oc == 1))
                    for b in range(g):
                        ex = expp.tile([P, iblk], AVT, tag="ex", name=f"ex{b}")
                        nc.scalar.activation(ex[:], scs[b][:], Exp)
                        exs[b].append(ex)
                    for ic in range(ICH):
                        ics = slice(ic * P, (ic + 1) * P)
                        nc.tensor.matmul(att_ps[ic][:], exs[0][jc][:, ics],
                                         vt[jc][:],
                                         start=(jc == 0), stop=(jc == JC - 1))
                normalize(att_ps, blk)
                for b in range(1, g):
                    att_psb = [psatt.tile([P, H + 2], f32, tag="att",
                                          name=f"attps{b}")
                               for _ in range(ICH)]
                    av_sweep(att_psb, exs[b], blk + b)
                    normalize(att_psb, blk + b)
                blk += g

    nc.compile()
    return nc


_NC_CACHE = {}


def _get_nc(nkv=N, nq=NQ, iblk=512):
    key = (nkv, nq, iblk)
    if key not in _NC_CACHE:
        _NC_CACHE[key] = build_nc(*key)
    return _NC_CACHE[key]


def _make_in_maps(x, Wq, bq, Wk, bk, Wv):
    wqT = np.ascontiguousarray(Wq.T.astype(np.float16))
    wkT = np.ascontiguousarray(Wk.T.astype(np.float16))
    wvT = np.ascontiguousarray(Wv.T.astype(np.float16))
    bq2 = np.ascontiguousarray(bq.reshape(H, 1))
    bk2 = np.ascontiguousarray(bk.reshape(H, 1))
    x16 = x.astype(np.float16)
    xT = [np.ascontiguousarray(x16[b].T) for b in range(B)]
    in_maps = []
    for c in range(8):
        b, half = c // 2, c % 2
        in_maps.append({
            "xT": xT[b],
            "xqT": np.ascontiguousarray(x16[b, half * NQ:(half + 1) * NQ, :].T),
            "wqT": wqT, "wkT": wkT, "wvT": wvT,
            "bq": bq2, "bk": bk2,
        })
    return in_maps


def _run(inputs, trace=False):
    from concourse.bass_utils import run_bass_kernel_spmd

    x = np.asarray(inputs["x"], dtype=np.float32)
    Wq = np.asarray(inputs["Wq"], dtype=np.float32)
    bq = np.asarray(inputs["bq"], dtype=np.float32)
    Wk = np.asarray(inputs["Wk"], dtype=np.float32)
    bk = np.asarray(inputs["bk"], dtype=np.float32)
    Wv = np.asarray(inputs["Wv"], dtype=np.float32)
    bv = np.asarray(inputs["bv"], dtype=np.float32)

    in_maps = _make_in_maps(x, Wq, bq, Wk, bk, Wv)
    # The device occasionally wedges on the first execution of a fresh
    # NEFF (NRT_EXEC_UNIT_UNRECOVERABLE); a retry with a slightly
    # perturbed program (different walrus schedule) recovers.
    last_exc = None
    for attempt in range(3):
        try:
            nc = _get_nc() if attempt == 0 else build_nc(salt=attempt)
            res = run_bass_kernel_spmd(nc, in_maps, list(range(8)), trace=trace)
            break
        except Exception as e:  # noqa: BLE001
            last_exc = e
            import os as _os
            import time as _time
            _os.environ["NEURON_RT_RESET_CORES"] = "1"
            _time.sleep(3)
    else:
        raise last_exc

    out = np.empty((B, N, H), dtype=np.float32)
    for c in range(8):
        b, half = c // 2, c % 2
        out[b, half * NQ:(half + 1) * NQ, :] = res.results[c]["att"] + bv
    return out, res


def kernel(**inputs) -> np.ndarray:
    out, _ = _run(inputs, trace=False)
    return out
